# revision 28
# baseline (speedup 1.0000x reference)
"""Trainium2 Bass kernel for nn_MultiHeadAttention_18425409700485.

B=2, S=2048, D=1024, H=16 heads (DH=64). 8 NeuronCores:
core c handles batch b = c // 4 and head group hg = c % 4 (4 heads each).

Reference semantics (deliberate quirks faithfully reproduced):
  q = query @ Wq ; k = key @ Wk ; v = value @ Wv           (biases are zero)
  scores = q k^T per head; causal mask of -1e9 added BEFORE dividing by
  sqrt(D)=32; softmax; x = attn @ v  [B,H,S,DH]
  "buggy" merge: x.swapaxes(-1,-2).reshape(B,-1,D) -> merged rows
  R = h*128 + 2*dh + t hold x[t*1024 + c, dh] at column c.
  out = merged @ Wo.  Heads map to disjoint output rows -> no collective.

v4 dataflow:
  * Host pre-transposes x -> x^T and casts x/W to bf16: every device DMA
    is a plain contiguous load (no SWDGE cast, no input DMA-transpose).
  * First s-block + qkv weights load on the sync HWDGE ring; the bulk
    (s-blocks 1-3, Wo) loads on the gpsimd SWDGE queue, gated behind the
    first block by a dummy data-dependency so they don't steal HBM
    bandwidth from the critical first block.  The scalar queue carries
    ONLY the ACTIVATE stream (a dma_start occupies its queue for the
    whole transfer and would stall exp).
  * Projections contract d on partitions straight out of x^T tiles.
  * Scores run quadrant-packed: per head pair, K=64 row groups x 64-col
    groups give 4 concurrent matmuls in disjoint PE-array quadrants.
  * Causal diagonal trimmed at 128-query granularity (score MMs, ACT
    spans, AV MMs restricted to the live span; exp'd tiles masked by a
    0/1 lower-triangle mask; stale-but-finite trimmed columns are
    multiplied to exact 0 and never consumed).
  * x_unnorm^T [65,512] tiles go natural-side via the DMA xbar (sync
    ring, 80-row slices for the 16-row xbar granule); normalization
    (1/denom) is applied by DVE while scattering into the buggy-merge
    layout of xall.
  * Emission weaves projections for s-block ic+1, finalizes, and the
    output projection between the ACT-bound attention groups via filler
    callbacks so the in-order PE queue never waits on exp.
"""

import os
import sys

sys.path.insert(0, "/opt/trn_rl_repo")

import numpy as np

S = 2048
D = 1024
H_PER_CORE = 4
DH = 64
SCALE = 1.0 / 32.0  # 1/sqrt(D)

_CACHE = {}


def _build_kernel():
    import concourse.bass as bass
    import concourse.mybir as mybir
    import concourse.tile as tile
    from concourse import bacc
    from contextlib import ExitStack

    fp32 = mybir.dt.float32
    bf16 = mybir.dt.bfloat16

    nc = bacc.Bacc("TRN2", target_bir_lowering=False, debug=False,
                   enable_asserts=False)

    xqT = nc.dram_tensor("xqT", [D, S], bf16, kind="ExternalInput").ap()
    xkT = nc.dram_tensor("xkT", [D, S], bf16, kind="ExternalInput").ap()
    xvT = nc.dram_tensor("xvT", [D, S], bf16, kind="ExternalInput").ap()
    wq = nc.dram_tensor("wq", [D, 256], bf16, kind="ExternalInput").ap()
    wk = nc.dram_tensor("wk", [D, 256], bf16, kind="ExternalInput").ap()
    wv = nc.dram_tensor("wv", [D, 256], bf16, kind="ExternalInput").ap()
    wo = nc.dram_tensor("wo", [D, D], bf16, kind="ExternalInput").ap()
    out = nc.dram_tensor("out", [512, D], fp32, kind="ExternalOutput").ap()

    Exp = mybir.ActivationFunctionType.Exp

    with tile.TileContext(nc) as tc, ExitStack() as ctx:
        const = ctx.enter_context(tc.tile_pool(name="const", bufs=1))
        persist = ctx.enter_context(tc.tile_pool(name="persist", bufs=1))
        # PSUM: "ps" [128,1024] tiles (2 banks) serve projections AND
        # attention scores; "xps" [128,512] tiles (1 bank) serve the AV
        # accumulator and the output projection.  2*2 + 4*1 = 8 banks.
        spsum = ctx.enter_context(tc.tile_pool(name="spsum", bufs=2,
                                               space="PSUM"))
        xps = ctx.enter_context(tc.tile_pool(name="xps", bufs=4,
                                             space="PSUM"))
        ptile = ctx.enter_context(tc.tile_pool(name="ptile", bufs=5))
        xtp = ctx.enter_context(tc.tile_pool(name="xtp", bufs=3))
        xtnp = ctx.enter_context(tc.tile_pool(name="xtnp", bufs=5))
        misc = ctx.enter_context(tc.tile_pool(name="misc", bufs=2))
        outp = ctx.enter_context(tc.tile_pool(name="outp", bufs=2))

        # --- constants -----------------------------------------------------
        mask4 = const.tile([128, 4, 512], bf16, name="mask4")
        nc.gpsimd.memset(mask4[:], 1.0)
        for o in range(4):
            nc.gpsimd.affine_select(
                out=mask4[:, o, :], in_=mask4[:, o, :],
                compare_op=mybir.AluOpType.is_ge, fill=0.0, base=-128 * o,
                pattern=[[1, 512]], channel_multiplier=-1)

        wq_sb = const.tile([128, 8, 256], bf16, name="wq_sb")
        wk_sb = const.tile([128, 8, 256], bf16, name="wk_sb")
        wv_sb = const.tile([128, 8, 256], bf16, name="wv_sb")
        wo_sb = const.tile([128, 8, 1024], bf16, name="wo_sb")

        xqT_sb = persist.tile([128, 8, S], bf16, name="xqT_sb")
        xkT_sb = persist.tile([128, 8, S], bf16, name="xkT_sb")
        xvT_sb = persist.tile([128, 8, S], bf16, name="xvT_sb")

        qT = persist.tile([128, 2, S], bf16, name="qT")
        kT = persist.tile([128, 2, S], bf16, name="kT")
        v65 = persist.tile([128, 16, 4 * 65], bf16, name="v65")
        nc.gpsimd.memset(
            v65.rearrange("p t (h c) -> p t h c", c=65)[:, :, :, 64], 1.0)
        xall = persist.tile([128, H_PER_CORE, 8, 128], bf16, name="xall")
        gate = const.tile([1, 16], bf16, name="gate")

        # --- DMA loads ----------------------------------------------------
        def load_block(eng, dram_ap, dst, i):
            eng.dma_start(
                dst[:, :, 512 * i:512 * (i + 1)],
                dram_ap[:, 512 * i:512 * (i + 1)].rearrange(
                    "(dc p) s -> p dc s", p=128))

        nc.sync.dma_start(wq_sb[:], wq.rearrange("(o p) m -> p o m", p=128))
        nc.sync.dma_start(wk_sb[:], wk.rearrange("(o p) m -> p o m", p=128))
        load_block(nc.sync, xqT, xqT_sb, 0)
        load_block(nc.sync, xkT, xkT_sb, 0)
        nc.sync.dma_start(wv_sb[:], wv.rearrange("(o p) m -> p o m", p=128))
        load_block(nc.sync, xvT, xvT_sb, 0)
        # Gate: the scheduler may hoist dependency-free DMAs, so each bulk
        # load is held back by a REAL hazard: a 1-element gpsimd op reads
        # (a) the chain value, whose root RAW-depends on the last
        # first-block load, and (b) one element of the bulk load's own
        # destination region — the DMA write then has a WAR dependency and
        # cannot start before the first block has the HBM to itself.
        nc.gpsimd.tensor_copy(gate[:1, 0:1], xvT_sb[:1, 0, 511:512])
        gi = [0]

        def gated(load_fn, dst_probe):
            gi[0] += 1
            nc.gpsimd.tensor_tensor(gate[:1, gi[0]:gi[0] + 1],
                                    gate[:1, 0:1], dst_probe,
                                    op=mybir.AluOpType.add)
            load_fn()

        for i in range(1, 4):
            for dram_ap, dst in ((xqT, xqT_sb), (xkT, xkT_sb),
                                 (xvT, xvT_sb)):
                gated(lambda d=dram_ap, t=dst, ii=i:
                      load_block(nc.gpsimd, d, t, ii),
                      dst[:1, 0, 512 * i:512 * i + 1])
        gated(lambda: nc.gpsimd.dma_start(
            wo_sb[:], wo.rearrange("(o p) m -> p o m", p=128)),
            wo_sb[:1, 0, 0:1])

        # --- building blocks ----------------------------------------------
        def qkproj(w_sb, x_sb, dst, a, ic, tag):
            """dst[:, a, 512*ic:...] = (W[:, 128a:128(a+1)])^T @ x^T block."""
            ps = spsum.tile([128, 1024], fp32, tag="ps",
                            name=f"pp_{tag}_{a}_{ic}")
            for dc in range(8):
                nc.tensor.matmul(
                    ps[:, :512],
                    lhsT=w_sb[:, dc, 128 * a:128 * (a + 1)],
                    rhs=x_sb[:, dc, 512 * ic:512 * (ic + 1)],
                    start=(dc == 0), stop=(dc == 7))
            nc.vector.tensor_copy(dst[:, a, 512 * ic:512 * (ic + 1)],
                                  ps[:, :512])

        def vproj(t):
            """v65[:, t, h*65:(h*65+64)] = x_v s-tile t @ Wv (natural)."""
            ps = spsum.tile([128, 1024], fp32, tag="ps", name=f"psv_{t}")
            for dc in range(8):
                nc.tensor.matmul(
                    ps[:, :256],
                    lhsT=xvT_sb[:, dc, 128 * t:128 * (t + 1)],
                    rhs=wv_sb[:, dc, :],
                    start=(dc == 0), stop=(dc == 7))
            nc.vector.tensor_copy(
                v65.rearrange("p t (h c) -> p t h c", c=65)[:, t, :, :64],
                ps[:, :256].rearrange("p (h c) -> p h c", c=64))

        def attn_pair(a, ic, fillers):
            """Scores+exp+mask+AV for heads (2a, 2a+1), queries block ic.

            Scores are quadrant-packed: head sg in PE rows 64sg..64sg+63,
            column group c in PE cols 64c..64c+63 -> 4 concurrent MMs.
            One filler callback is drained after each b2 group so the PE
            has work while ACT churns through exp.
            Returns the two xtn tiles (transposed unnormalized x+denom).
            """
            nlive = 4 * (ic + 1)
            nbatch = nlive // 2
            px = [xps.tile([128, 512], fp32, tag="xps",
                           name=f"px_{a}_{ic}_{sg}") for sg in range(2)]
            pbs = [None] * nbatch
            xtns = []
            for b2 in range(nbatch + 1):
                if b2 < nbatch:
                    diag = 2 * b2 >= 4 * ic
                    pss = [spsum.tile([128, 1024], fp32, tag="ps",
                                      name=f"ps_{a}_{ic}_{b2}_{sg}")
                           for sg in range(2)]
                    qo0 = 0
                    for k2 in range(2):
                        jj = 2 * b2 + k2
                        o = jj - 4 * ic
                        qo = 128 * o if o > 0 else 0
                        if k2 == 0:
                            qo0 = qo
                        for sg in range(2):
                            po = 64 * sg
                            nc.tensor.matmul(
                                pss[sg][:, 512 * k2 + qo:512 * (k2 + 1)],
                                lhsT=kT[po:po + 64, a,
                                        128 * jj:128 * (jj + 1)],
                                rhs=qT[po:po + 64, a,
                                       512 * ic + qo:512 * (ic + 1)],
                                start=True, stop=True)
                    pbp = [ptile.tile([128, 2, 512], bf16, tag="pb",
                                      name=f"pb_{a}_{ic}_{b2}_{sg}")
                           for sg in range(2)]
                    for sg in range(2):
                        pb2d = pbp[sg].rearrange("p k f -> p (k f)")
                        # exp over [first live col of k2=0 .. end]; the
                        # dead middle columns hold stale-but-finite fp32,
                        # get multiplied to 0 by the mask, and the AV MMs
                        # never consume them.
                        nc.scalar.activation(pb2d[:, qo0:], pss[sg][:, qo0:],
                                             Exp, scale=SCALE)
                        if diag:
                            o0 = 2 * b2 - 4 * ic
                            nc.vector.tensor_mul(
                                pb2d[:, qo0:], pb2d[:, qo0:],
                                mask4[:, o0:o0 + 2, :].rearrange(
                                    "p k f -> p (k f)")[:, qo0:])
                    pbs[b2] = pbp
                if b2 >= 1:
                    for k2 in range(2):
                        jj = 2 * (b2 - 1) + k2
                        o = jj - 4 * ic
                        qo = 128 * o if o > 0 else 0
                        for sg in range(2):
                            h = 2 * a + sg
                            nc.tensor.matmul(
                                px[sg][:65, qo:],
                                lhsT=v65[:, jj, 65 * h:65 * (h + 1)],
                                rhs=pbs[b2 - 1][sg][:, k2, qo:],
                                start=(jj == 0), stop=(jj == nlive - 1))
                if fillers:
                    fillers.pop(0)()
            for sg in range(2):
                h = 2 * a + sg
                # x_unnorm^T (+denom row 64) -> natural via DMA xbar.
                # 80 = next multiple of XBAR_TILE_SRC_ROWS(16) above 65.
                xt_sb = xtp.tile([80, 512], bf16, tag="xt",
                                 name=f"xt_{h}_{ic}")
                nc.vector.tensor_copy(xt_sb[:65, :], px[sg][:65, :])
                xtn = xtnp.tile([128, 4, 80], bf16, tag="xtn",
                                name=f"xtn_{h}_{ic}")
                nc.sync.dma_start(xtn[:], xt_sb[:], transpose=True)
                xtns.append(xtn)
            return xtns

        def finalize(h, ic, xtn):
            """1/denom scatter into the buggy-merge layout of xall."""
            recip4 = misc.tile([128, 4], fp32, tag="recip4",
                               name=f"rc_{h}_{ic}")
            nc.vector.reciprocal(recip4[:], xtn[:, :, 64])
            for k4 in range(4):
                j = 4 * ic + k4
                nc.vector.tensor_scalar_mul(
                    xall[:, h, j % 8, (j // 8)::2],
                    xtn[:, k4, :64], recip4[:, k4:k4 + 1])

        def outproj(h):
            ot = outp.tile([128, 2, 512], fp32, tag="ot", name=f"ot_{h}")
            for nn in range(2):
                po_ = xps.tile([128, 512], fp32, tag="xps",
                               name=f"po_{h}_{nn}")
                for q8 in range(8):
                    nc.tensor.matmul(
                        po_[:],
                        lhsT=xall[:, h, q8, :],
                        rhs=wo_sb[:, q8, 512 * nn:512 * (nn + 1)],
                        start=(q8 == 0), stop=(q8 == 7))
                nc.vector.tensor_copy(ot[:, nn, :], po_[:])
            nc.sync.dma_start(out[128 * h:128 * (h + 1), :],
                              ot.rearrange("p k f -> p (k f)"))

        # === emission: PE queue order == program order.  Projections for
        # s-block ic+1, deferred finalizes and the output projection are
        # woven between the ACT-bound attention groups. ===================
        qkproj(wq_sb, xqT_sb, qT, 0, 0, "q")
        qkproj(wq_sb, xqT_sb, qT, 1, 0, "q")
        qkproj(wk_sb, xkT_sb, kT, 0, 0, "k")
        qkproj(wk_sb, xkT_sb, kT, 1, 0, "k")
        for t in range(4):
            vproj(t)

        # Deadline-aware filler quotas per (a, ic) slot.  qkproj(n) must be
        # fully emitted before pair(0, n) starts (its scores read qT/kT in
        # program order); vproj(t) before the AV matmuls of j-tile t
        # (earliest use: slot (0, t//4) at b2 >= 2*(t//4), so early slots
        # of that same pair are legal); outproj(h) after finalize(h, 3).
        def Q(w, x, dst, aa, n, tag):
            return lambda: qkproj(w, x, dst, aa, n, tag)

        def V(t):
            return lambda: vproj(t)

        slot_work = {
            (0, 0): [Q(wq_sb, xqT_sb, qT, 0, 1, "q"),
                     Q(wq_sb, xqT_sb, qT, 1, 1, "q"),
                     Q(wk_sb, xkT_sb, kT, 0, 1, "k")],
            (1, 0): [Q(wk_sb, xkT_sb, kT, 1, 1, "k"), V(4), V(5)],
            (0, 1): [V(6), V(7),
                     Q(wq_sb, xqT_sb, qT, 0, 2, "q"),
                     Q(wq_sb, xqT_sb, qT, 1, 2, "q"),
                     Q(wk_sb, xkT_sb, kT, 0, 2, "k")],
            (1, 1): [Q(wk_sb, xkT_sb, kT, 1, 2, "k"),
                     V(8), V(9), V(10), V(11)],
            (0, 2): [Q(wq_sb, xqT_sb, qT, 0, 3, "q"),
                     Q(wq_sb, xqT_sb, qT, 1, 3, "q"),
                     Q(wk_sb, xkT_sb, kT, 0, 3, "k"),
                     Q(wk_sb, xkT_sb, kT, 1, 3, "k")],
            (1, 2): [V(14), V(15)],
            (0, 3): [V(12), V(13)],
            (1, 3): [],
        }

        # Finalizes defer one pair-block normally; ic=0 blocks are so
        # short that their transpose+reciprocal would still sit on the
        # DVE critical path, so those defer two pair-blocks.
        pending = []  # [(h, ic, xtn, age), ...]
        for ic in range(4):
            for a in range(2):
                pending = [(h, i, x, age + 1) for (h, i, x, age) in pending]
                need = lambda p: p[3] >= (2 if p[1] == 0 else 1)
                drain = [p for p in pending if need(p)]
                pending = [p for p in pending if not need(p)]
                fillers = []
                for (hp, icp, xtnp_, _) in drain:
                    fillers.append(
                        lambda hp=hp, icp=icp, x=xtnp_: finalize(hp, icp, x))
                for (hp, icp, xtnp_, _) in drain:
                    if icp == 3:
                        fillers.append(lambda hp=hp: outproj(hp))
                fillers.extend(slot_work[(a, ic)])
                xtns = attn_pair(a, ic, fillers)
                for f in fillers:  # leftovers (early ic: fewer b2 slots)
                    f()
                fillers.clear()
                pending.append((2 * a, ic, xtns[0], 0))
                pending.append((2 * a + 1, ic, xtns[1], 0))
        for (hp, icp, xtnp_, _) in pending:
            finalize(hp, icp, xtnp_)
            outproj(hp)

    nc.compile()
    return nc


def _get_nc():
    if "nc" not in _CACHE:
        _CACHE["nc"] = _build_kernel()
    return _CACHE["nc"]


def kernel(query, key, value, Wq, bq, Wk, bk, Wv, bv, Wo, bo):
    """Full inputs in, full output out. Shards batch x head-group over 8
    cores; host pre-transposes/casts so device DMA is pure bf16 HWDGE."""
    nc = _get_nc()
    from concourse.bass_utils import run_bass_kernel_spmd
    import ml_dtypes

    bf16 = ml_dtypes.bfloat16
    query = np.asarray(query, dtype=np.float32)
    key = np.asarray(key, dtype=np.float32)
    value = np.asarray(value, dtype=np.float32)
    Wq = np.asarray(Wq, dtype=np.float32)
    Wk = np.asarray(Wk, dtype=np.float32)
    Wv = np.asarray(Wv, dtype=np.float32)
    Wo = np.asarray(Wo, dtype=np.float32)

    B = query.shape[0]
    xqT = [np.ascontiguousarray(query[b].T).astype(bf16) for b in range(B)]
    xkT = [np.ascontiguousarray(key[b].T).astype(bf16) for b in range(B)]
    xvT = [np.ascontiguousarray(value[b].T).astype(bf16) for b in range(B)]
    wo_b = Wo.astype(bf16)

    in_maps = []
    for c in range(8):
        b, hg = c // 4, c % 4
        cols = slice(256 * hg, 256 * (hg + 1))
        in_maps.append({
            "xqT": xqT[b],
            "xkT": xkT[b],
            "xvT": xvT[b],
            "wq": np.ascontiguousarray(Wq[:, cols]).astype(bf16),
            "wk": np.ascontiguousarray(Wk[:, cols]).astype(bf16),
            "wv": np.ascontiguousarray(Wv[:, cols]).astype(bf16),
            "wo": wo_b,
        })

    trace = bool(int(os.environ.get("KERNEL_TRACE", "0")))
    res = run_bass_kernel_spmd(nc, in_maps, core_ids=list(range(8)),
                               trace=trace)
    _CACHE["last_result"] = res

    full = np.zeros((B, S, D), dtype=np.float32)
    for c in range(8):
        b, hg = c // 4, c % 4
        full[b, 512 * hg:512 * (hg + 1), :] = res.results[c]["out"]
    return full


# revision 36
# speedup vs baseline: 1.0774x; 1.0774x over previous
"""Trainium2 Bass kernel for nn_MultiHeadAttention_18425409700485.

B=2, S=2048, D=1024, H=16 heads (DH=64). 8 NeuronCores:
core c handles batch b = c // 4 and head group hg = c % 4 (4 heads each).

Reference semantics (deliberate quirks faithfully reproduced):
  q = query @ Wq ; k = key @ Wk ; v = value @ Wv           (biases are zero)
  scores = q k^T per head; causal mask of -1e9 added BEFORE dividing by
  sqrt(D)=32; softmax; x = attn @ v  [B,H,S,DH]
  "buggy" merge: x.swapaxes(-1,-2).reshape(B,-1,D) -> merged rows
  R = h*128 + 2*dh + t hold x[t*1024 + c, dh] at column c.
  out = merged @ Wo.  Heads map to disjoint output rows -> no collective.

v4 dataflow:
  * Host pre-transposes x -> x^T and casts x/W to bf16: every device DMA
    is a plain contiguous load (no SWDGE cast, no input DMA-transpose).
  * First s-block + qkv weights load on the sync HWDGE ring; the bulk
    (s-blocks 1-3, Wo) loads on the gpsimd SWDGE queue, gated behind the
    first block by a dummy data-dependency so they don't steal HBM
    bandwidth from the critical first block.  The scalar queue carries
    ONLY the ACTIVATE stream (a dma_start occupies its queue for the
    whole transfer and would stall exp).
  * Projections contract d on partitions straight out of x^T tiles.
  * Scores run quadrant-packed: per head pair, K=64 row groups x 64-col
    groups give 4 concurrent matmuls in disjoint PE-array quadrants.
  * Causal diagonal trimmed at 128-query granularity (score MMs, ACT
    spans, AV MMs restricted to the live span; exp'd tiles masked by a
    0/1 lower-triangle mask; stale-but-finite trimmed columns are
    multiplied to exact 0 and never consumed).
  * x_unnorm^T [65,512] tiles go natural-side via the DMA xbar (sync
    ring, 80-row slices for the 16-row xbar granule); normalization
    (1/denom) is applied by DVE while scattering into the buggy-merge
    layout of xall.
  * Emission weaves projections for s-block ic+1, finalizes, and the
    output projection between the ACT-bound attention groups via filler
    callbacks so the in-order PE queue never waits on exp.
"""

import os
import sys

sys.path.insert(0, "/opt/trn_rl_repo")

import numpy as np

S = 2048
D = 1024
H_PER_CORE = 4
DH = 64
SCALE = 1.0 / 32.0  # 1/sqrt(D)

_CACHE = {}


def _build_kernel():
    import concourse.bass as bass
    import concourse.mybir as mybir
    import concourse.tile as tile
    from concourse import bacc
    from contextlib import ExitStack

    fp32 = mybir.dt.float32
    bf16 = mybir.dt.bfloat16

    nc = bacc.Bacc("TRN2", target_bir_lowering=False, debug=False,
                   enable_asserts=False)

    # All inputs arrive host-pre-tiled in the exact SBUF layout so every
    # load is fully linear on BOTH the HBM and SBUF side (strided loads
    # measured only ~210 GB/s; linear ~400 GB/s).
    # x^T blocks: [i(s-block), p, dc, s_local] with d = dc*128+p,
    # s = 512*i + s_local.  Weights: [p, o, m] with row = o*128+p.
    xqT = nc.dram_tensor("xqT", [4, 128, 8, 512], bf16,
                         kind="ExternalInput").ap()
    xkT = nc.dram_tensor("xkT", [4, 128, 8, 512], bf16,
                         kind="ExternalInput").ap()
    xvT = nc.dram_tensor("xvT", [4, 128, 8, 512], bf16,
                         kind="ExternalInput").ap()
    wq = nc.dram_tensor("wq", [128, 8, 256], bf16, kind="ExternalInput").ap()
    wk = nc.dram_tensor("wk", [128, 8, 256], bf16, kind="ExternalInput").ap()
    wv = nc.dram_tensor("wv", [128, 8, 256], bf16, kind="ExternalInput").ap()
    wo = nc.dram_tensor("wo", [128, 8, 1024], bf16,
                        kind="ExternalInput").ap()
    out = nc.dram_tensor("out", [512, D], fp32, kind="ExternalOutput").ap()

    Exp = mybir.ActivationFunctionType.Exp

    with tile.TileContext(nc) as tc, ExitStack() as ctx:
        const = ctx.enter_context(tc.tile_pool(name="const", bufs=1))
        persist = ctx.enter_context(tc.tile_pool(name="persist", bufs=1))
        # PSUM: "ps" [128,1024] tiles (2 banks) serve projections AND
        # attention scores; "xps" [128,512] tiles (1 bank) serve the AV
        # accumulator and the output projection.  2*2 + 4*1 = 8 banks.
        spsum = ctx.enter_context(tc.tile_pool(name="spsum", bufs=2,
                                               space="PSUM"))
        xps = ctx.enter_context(tc.tile_pool(name="xps", bufs=4,
                                             space="PSUM"))
        ptile = ctx.enter_context(tc.tile_pool(name="ptile", bufs=5))
        xtp = ctx.enter_context(tc.tile_pool(name="xtp", bufs=3))
        xtnp = ctx.enter_context(tc.tile_pool(name="xtnp", bufs=5))
        misc = ctx.enter_context(tc.tile_pool(name="misc", bufs=2))
        outp = ctx.enter_context(tc.tile_pool(name="outp", bufs=2))

        # --- constants -----------------------------------------------------
        mask4 = const.tile([128, 4, 512], bf16, name="mask4")
        nc.gpsimd.memset(mask4[:], 1.0)
        for o in range(4):
            nc.gpsimd.affine_select(
                out=mask4[:, o, :], in_=mask4[:, o, :],
                compare_op=mybir.AluOpType.is_ge, fill=0.0, base=-128 * o,
                pattern=[[1, 512]], channel_multiplier=-1)

        wq_sb = const.tile([128, 8, 256], bf16, name="wq_sb")
        wk_sb = const.tile([128, 8, 256], bf16, name="wk_sb")
        wv_sb = const.tile([128, 8, 256], bf16, name="wv_sb")
        wo_sb = const.tile([128, 8, 1024], bf16, name="wo_sb")

        xqT_sb = persist.tile([128, 4, 8, 512], bf16, name="xqT_sb")
        xkT_sb = persist.tile([128, 4, 8, 512], bf16, name="xkT_sb")
        xvT_sb = persist.tile([128, 4, 8, 512], bf16, name="xvT_sb")

        qT = persist.tile([128, 2, S], bf16, name="qT")
        kT = persist.tile([128, 2, S], bf16, name="kT")
        v65 = persist.tile([128, 16, 4 * 65], bf16, name="v65")
        nc.gpsimd.memset(
            v65.rearrange("p t (h c) -> p t h c", c=65)[:, :, :, 64], 1.0)
        xall = persist.tile([128, H_PER_CORE, 8, 128], bf16, name="xall")
        gate = const.tile([1, 16], bf16, name="gate")

        # --- DMA loads ----------------------------------------------------
        def load_block(eng, dram_ap, dst, i):
            eng.dma_start(dst[:, i], dram_ap[i])

        nc.sync.dma_start(wq_sb[:], wq[:])
        nc.sync.dma_start(wk_sb[:], wk[:])
        load_block(nc.sync, xqT, xqT_sb, 0)
        load_block(nc.sync, xkT, xkT_sb, 0)
        nc.sync.dma_start(wv_sb[:], wv[:])
        load_block(nc.sync, xvT, xvT_sb, 0)
        # Gate: the scheduler may hoist dependency-free DMAs, so each bulk
        # load is held back by a REAL hazard: a 1-element gpsimd op reads
        # (a) the chain value, whose root RAW-depends on the last
        # first-block load, and (b) one element of the bulk load's own
        # destination region — the DMA write then has a WAR dependency and
        # cannot start before the first block has the HBM to itself.
        nc.gpsimd.tensor_copy(gate[:1, 0:1], xvT_sb[:1, 0, 0, 511:512])
        gi = [0]

        def gated(load_fn, dst_probe):
            gi[0] += 1
            nc.gpsimd.tensor_tensor(gate[:1, gi[0]:gi[0] + 1],
                                    gate[:1, 0:1], dst_probe,
                                    op=mybir.AluOpType.add)
            load_fn()

        for i in range(1, 4):
            for dram_ap, dst in ((xqT, xqT_sb), (xkT, xkT_sb),
                                 (xvT, xvT_sb)):
                gated(lambda d=dram_ap, t=dst, ii=i:
                      load_block(nc.gpsimd, d, t, ii),
                      dst[:1, i, 0, 0:1])
        gated(lambda: nc.gpsimd.dma_start(wo_sb[:], wo[:]),
              wo_sb[:1, 0, 0:1])

        # --- building blocks ----------------------------------------------
        def qkproj(w_sb, x_sb, dst, a, ic, tag):
            """dst[:, a, 512*ic:...] = (W[:, 128a:128(a+1)])^T @ x^T block."""
            ps = spsum.tile([128, 1024], fp32, tag="ps",
                            name=f"pp_{tag}_{a}_{ic}")
            for dc in range(8):
                nc.tensor.matmul(
                    ps[:, :512],
                    lhsT=w_sb[:, dc, 128 * a:128 * (a + 1)],
                    rhs=x_sb[:, ic, dc, :],
                    start=(dc == 0), stop=(dc == 7))
            nc.vector.tensor_copy(dst[:, a, 512 * ic:512 * (ic + 1)],
                                  ps[:, :512])

        def vproj(t):
            """v65[:, t, h*65:(h*65+64)] = x_v s-tile t @ Wv (natural)."""
            ps = spsum.tile([128, 1024], fp32, tag="ps", name=f"psv_{t}")
            for dc in range(8):
                nc.tensor.matmul(
                    ps[:, :256],
                    lhsT=xvT_sb[:, t // 4, dc,
                                128 * (t % 4):128 * (t % 4 + 1)],
                    rhs=wv_sb[:, dc, :],
                    start=(dc == 0), stop=(dc == 7))
            nc.vector.tensor_copy(
                v65.rearrange("p t (h c) -> p t h c", c=65)[:, t, :, :64],
                ps[:, :256].rearrange("p (h c) -> p h c", c=64))

        def attn_pair(a, ic, fillers):
            """Scores+exp+mask+AV for heads (2a, 2a+1), queries block ic.

            Scores are quadrant-packed: head sg in PE rows 64sg..64sg+63,
            column group c in PE cols 64c..64c+63 -> 4 concurrent MMs.
            One filler callback is drained after each b2 group so the PE
            has work while ACT churns through exp.
            Returns the two xtn tiles (transposed unnormalized x+denom).
            """
            nlive = 4 * (ic + 1)
            nbatch = nlive // 2
            px = [xps.tile([128, 512], fp32, tag="xps",
                           name=f"px_{a}_{ic}_{sg}") for sg in range(2)]
            pbs = [None] * nbatch
            xtns = []
            for b2 in range(nbatch + 1):
                if b2 < nbatch:
                    diag = 2 * b2 >= 4 * ic
                    pss = [spsum.tile([128, 1024], fp32, tag="ps",
                                      name=f"ps_{a}_{ic}_{b2}_{sg}")
                           for sg in range(2)]
                    qo0 = 0
                    for k2 in range(2):
                        jj = 2 * b2 + k2
                        o = jj - 4 * ic
                        qo = 128 * o if o > 0 else 0
                        if k2 == 0:
                            qo0 = qo
                        for sg in range(2):
                            po = 64 * sg
                            nc.tensor.matmul(
                                pss[sg][:, 512 * k2 + qo:512 * (k2 + 1)],
                                lhsT=kT[po:po + 64, a,
                                        128 * jj:128 * (jj + 1)],
                                rhs=qT[po:po + 64, a,
                                       512 * ic + qo:512 * (ic + 1)],
                                start=True, stop=True)
                    pbp = [ptile.tile([128, 2, 512], bf16, tag="pb",
                                      name=f"pb_{a}_{ic}_{b2}_{sg}")
                           for sg in range(2)]
                    for sg in range(2):
                        pb2d = pbp[sg].rearrange("p k f -> p (k f)")
                        # exp over [first live col of k2=0 .. end]; the
                        # dead middle columns hold stale-but-finite fp32,
                        # get multiplied to 0 by the mask, and the AV MMs
                        # never consume them.
                        nc.scalar.activation(pb2d[:, qo0:], pss[sg][:, qo0:],
                                             Exp, scale=SCALE)
                        if diag:
                            o0 = 2 * b2 - 4 * ic
                            nc.vector.tensor_mul(
                                pb2d[:, qo0:], pb2d[:, qo0:],
                                mask4[:, o0:o0 + 2, :].rearrange(
                                    "p k f -> p (k f)")[:, qo0:])
                    pbs[b2] = pbp
                if b2 >= 1:
                    for k2 in range(2):
                        jj = 2 * (b2 - 1) + k2
                        o = jj - 4 * ic
                        qo = 128 * o if o > 0 else 0
                        for sg in range(2):
                            h = 2 * a + sg
                            nc.tensor.matmul(
                                px[sg][:65, qo:],
                                lhsT=v65[:, jj, 65 * h:65 * (h + 1)],
                                rhs=pbs[b2 - 1][sg][:, k2, qo:],
                                start=(jj == 0), stop=(jj == nlive - 1))
                if fillers:
                    fillers.pop(0)()
            for sg in range(2):
                h = 2 * a + sg
                # x_unnorm^T (+denom row 64) -> natural via DMA xbar.
                # 80 = next multiple of XBAR_TILE_SRC_ROWS(16) above 65.
                xt_sb = xtp.tile([80, 512], bf16, tag="xt",
                                 name=f"xt_{h}_{ic}")
                nc.vector.tensor_copy(xt_sb[:65, :], px[sg][:65, :])
                xtn = xtnp.tile([128, 4, 80], bf16, tag="xtn",
                                name=f"xtn_{h}_{ic}")
                nc.sync.dma_start(xtn[:], xt_sb[:], transpose=True)
                xtns.append(xtn)
            return xtns

        def finalize(h, ic, xtn):
            """1/denom scatter into the buggy-merge layout of xall."""
            recip4 = misc.tile([128, 4], fp32, tag="recip4",
                               name=f"rc_{h}_{ic}")
            nc.vector.reciprocal(recip4[:], xtn[:, :, 64])
            for k4 in range(4):
                j = 4 * ic + k4
                nc.vector.tensor_scalar_mul(
                    xall[:, h, j % 8, (j // 8)::2],
                    xtn[:, k4, :64], recip4[:, k4:k4 + 1])

        def outproj(h):
            ot = outp.tile([128, 2, 512], fp32, tag="ot", name=f"ot_{h}")
            for nn in range(2):
                po_ = xps.tile([128, 512], fp32, tag="xps",
                               name=f"po_{h}_{nn}")
                for q8 in range(8):
                    nc.tensor.matmul(
                        po_[:],
                        lhsT=xall[:, h, q8, :],
                        rhs=wo_sb[:, q8, 512 * nn:512 * (nn + 1)],
                        start=(q8 == 0), stop=(q8 == 7))
                nc.vector.tensor_copy(ot[:, nn, :], po_[:])
            nc.sync.dma_start(out[128 * h:128 * (h + 1), :],
                              ot.rearrange("p k f -> p (k f)"))

        # === emission: PE queue order == program order.  Projections for
        # s-block ic+1, deferred finalizes and the output projection are
        # woven between the ACT-bound attention groups. ===================
        qkproj(wq_sb, xqT_sb, qT, 0, 0, "q")
        qkproj(wq_sb, xqT_sb, qT, 1, 0, "q")
        qkproj(wk_sb, xkT_sb, kT, 0, 0, "k")
        qkproj(wk_sb, xkT_sb, kT, 1, 0, "k")
        for t in range(4):
            vproj(t)

        # Deadline-aware filler quotas per (a, ic) slot.  qkproj(n) must be
        # fully emitted before pair(0, n) starts (its scores read qT/kT in
        # program order); vproj(t) before the AV matmuls of j-tile t
        # (earliest use: slot (0, t//4) at b2 >= 2*(t//4), so early slots
        # of that same pair are legal); outproj(h) after finalize(h, 3).
        def Q(w, x, dst, aa, n, tag):
            return lambda: qkproj(w, x, dst, aa, n, tag)

        def V(t):
            return lambda: vproj(t)

        slot_work = {
            (0, 0): [Q(wq_sb, xqT_sb, qT, 0, 1, "q"),
                     Q(wq_sb, xqT_sb, qT, 1, 1, "q"),
                     Q(wk_sb, xkT_sb, kT, 0, 1, "k")],
            (1, 0): [Q(wk_sb, xkT_sb, kT, 1, 1, "k"), V(4), V(5)],
            (0, 1): [V(6), V(7),
                     Q(wq_sb, xqT_sb, qT, 0, 2, "q"),
                     Q(wq_sb, xqT_sb, qT, 1, 2, "q"),
                     Q(wk_sb, xkT_sb, kT, 0, 2, "k")],
            (1, 1): [Q(wk_sb, xkT_sb, kT, 1, 2, "k"),
                     V(8), V(9), V(10), V(11)],
            (0, 2): [Q(wq_sb, xqT_sb, qT, 0, 3, "q"),
                     Q(wq_sb, xqT_sb, qT, 1, 3, "q"),
                     Q(wk_sb, xkT_sb, kT, 0, 3, "k"),
                     Q(wk_sb, xkT_sb, kT, 1, 3, "k")],
            (1, 2): [V(14), V(15)],
            (0, 3): [V(12), V(13)],
            (1, 3): [],
        }

        # Finalizes defer one pair-block normally; ic=0 blocks are so
        # short that their transpose+reciprocal would still sit on the
        # DVE critical path, so those defer two pair-blocks.
        pending = []  # [(h, ic, xtn, age), ...]
        for ic in range(4):
            for a in range(2):
                pending = [(h, i, x, age + 1) for (h, i, x, age) in pending]
                need = lambda p: p[3] >= (2 if p[1] == 0 else 1)
                drain = [p for p in pending if need(p)]
                pending = [p for p in pending if not need(p)]
                fillers = []
                for (hp, icp, xtnp_, _) in drain:
                    fillers.append(
                        lambda hp=hp, icp=icp, x=xtnp_: finalize(hp, icp, x))
                for (hp, icp, xtnp_, _) in drain:
                    if icp == 3:
                        fillers.append(lambda hp=hp: outproj(hp))
                fillers.extend(slot_work[(a, ic)])
                xtns = attn_pair(a, ic, fillers)
                for f in fillers:  # leftovers (early ic: fewer b2 slots)
                    f()
                fillers.clear()
                pending.append((2 * a, ic, xtns[0], 0))
                pending.append((2 * a + 1, ic, xtns[1], 0))
        for (hp, icp, xtnp_, _) in pending:
            finalize(hp, icp, xtnp_)
            outproj(hp)

    nc.compile()
    return nc


def _get_nc():
    if "nc" not in _CACHE:
        _CACHE["nc"] = _build_kernel()
    return _CACHE["nc"]


def kernel(query, key, value, Wq, bq, Wk, bk, Wv, bv, Wo, bo):
    """Full inputs in, full output out. Shards batch x head-group over 8
    cores; host pre-transposes/casts so device DMA is pure bf16 HWDGE."""
    nc = _get_nc()
    from concourse.bass_utils import run_bass_kernel_spmd
    import ml_dtypes

    bf16 = ml_dtypes.bfloat16
    query = np.asarray(query, dtype=np.float32)
    key = np.asarray(key, dtype=np.float32)
    value = np.asarray(value, dtype=np.float32)
    Wq = np.asarray(Wq, dtype=np.float32)
    Wk = np.asarray(Wk, dtype=np.float32)
    Wv = np.asarray(Wv, dtype=np.float32)
    Wo = np.asarray(Wo, dtype=np.float32)

    # Pre-tile on host into the exact SBUF layouts so device DMAs are
    # fully linear: x^T -> [i, p, dc, s_local]; W -> [p, o, m].
    def tile_xT(x):  # x: [S, D] fp32 -> [4, 128, 8, 512] bf16
        xT = x.T.astype(bf16)  # [D, S]
        return np.ascontiguousarray(
            xT.reshape(8, 128, 4, 512).transpose(2, 1, 0, 3))

    def tile_w(w):  # w: [D, M] fp32 -> [128, 8, M] bf16
        m = w.shape[1]
        return np.ascontiguousarray(
            w.astype(bf16).reshape(8, 128, m).transpose(1, 0, 2))

    B = query.shape[0]
    xqT = [tile_xT(query[b]) for b in range(B)]
    xkT = [tile_xT(key[b]) for b in range(B)]
    xvT = [tile_xT(value[b]) for b in range(B)]
    wo_b = tile_w(Wo)

    in_maps = []
    for c in range(8):
        b, hg = c // 4, c % 4
        cols = slice(256 * hg, 256 * (hg + 1))
        in_maps.append({
            "xqT": xqT[b],
            "xkT": xkT[b],
            "xvT": xvT[b],
            "wq": tile_w(np.ascontiguousarray(Wq[:, cols])),
            "wk": tile_w(np.ascontiguousarray(Wk[:, cols])),
            "wv": tile_w(np.ascontiguousarray(Wv[:, cols])),
            "wo": wo_b,
        })

    trace = bool(int(os.environ.get("KERNEL_TRACE", "0")))
    res = run_bass_kernel_spmd(nc, in_maps, core_ids=list(range(8)),
                               trace=trace)
    _CACHE["last_result"] = res

    full = np.zeros((B, S, D), dtype=np.float32)
    for c in range(8):
        b, hg = c // 4, c % 4
        full[b, 512 * hg:512 * (hg + 1), :] = res.results[c]["out"]
    return full


# revision 37
# speedup vs baseline: 1.1370x; 1.0554x over previous
"""Trainium2 Bass kernel for nn_MultiHeadAttention_18425409700485.

B=2, S=2048, D=1024, H=16 heads (DH=64). 8 NeuronCores:
core c handles batch b = c // 4 and head group hg = c % 4 (4 heads each).

Reference semantics (deliberate quirks faithfully reproduced):
  q = query @ Wq ; k = key @ Wk ; v = value @ Wv           (biases are zero)
  scores = q k^T per head; causal mask of -1e9 added BEFORE dividing by
  sqrt(D)=32; softmax; x = attn @ v  [B,H,S,DH]
  "buggy" merge: x.swapaxes(-1,-2).reshape(B,-1,D) -> merged rows
  R = h*128 + 2*dh + t hold x[t*1024 + c, dh] at column c.
  out = merged @ Wo.  Heads map to disjoint output rows -> no collective.

v4 dataflow:
  * Host pre-transposes x -> x^T and casts x/W to bf16: every device DMA
    is a plain contiguous load (no SWDGE cast, no input DMA-transpose).
  * First s-block + qkv weights load on the sync HWDGE ring; the bulk
    (s-blocks 1-3, Wo) loads on the gpsimd SWDGE queue, gated behind the
    first block by a dummy data-dependency so they don't steal HBM
    bandwidth from the critical first block.  The scalar queue carries
    ONLY the ACTIVATE stream (a dma_start occupies its queue for the
    whole transfer and would stall exp).
  * Projections contract d on partitions straight out of x^T tiles.
  * Scores run quadrant-packed: per head pair, K=64 row groups x 64-col
    groups give 4 concurrent matmuls in disjoint PE-array quadrants.
  * Causal diagonal trimmed at 128-query granularity (score MMs, ACT
    spans, AV MMs restricted to the live span; exp'd tiles masked by a
    0/1 lower-triangle mask; stale-but-finite trimmed columns are
    multiplied to exact 0 and never consumed).
  * x_unnorm^T [65,512] tiles go natural-side via the DMA xbar (sync
    ring, 80-row slices for the 16-row xbar granule); normalization
    (1/denom) is applied by DVE while scattering into the buggy-merge
    layout of xall.
  * Emission weaves projections for s-block ic+1, finalizes, and the
    output projection between the ACT-bound attention groups via filler
    callbacks so the in-order PE queue never waits on exp.
"""

import os
import sys

sys.path.insert(0, "/opt/trn_rl_repo")

import numpy as np

S = 2048
D = 1024
H_PER_CORE = 4
DH = 64
SCALE = 1.0 / 32.0  # 1/sqrt(D)

_CACHE = {}


def _build_kernel():
    import concourse.bass as bass
    import concourse.mybir as mybir
    import concourse.tile as tile
    from concourse import bacc
    from contextlib import ExitStack

    fp32 = mybir.dt.float32
    bf16 = mybir.dt.bfloat16

    nc = bacc.Bacc("TRN2", target_bir_lowering=False, debug=False,
                   enable_asserts=False)

    # All inputs arrive host-pre-tiled in the exact SBUF layout so every
    # load is fully linear on BOTH the HBM and SBUF side (strided loads
    # measured only ~210 GB/s; linear ~400 GB/s).
    # x^T blocks: [i(s-block), p, dc, s_local] with d = dc*128+p,
    # s = 512*i + s_local.  Weights: [p, o, m] with row = o*128+p.
    xqT = nc.dram_tensor("xqT", [4, 128, 8, 512], bf16,
                         kind="ExternalInput").ap()
    xkT = nc.dram_tensor("xkT", [4, 128, 8, 512], bf16,
                         kind="ExternalInput").ap()
    xvT = nc.dram_tensor("xvT", [4, 128, 8, 512], bf16,
                         kind="ExternalInput").ap()
    wq = nc.dram_tensor("wq", [128, 8, 256], bf16, kind="ExternalInput").ap()
    wk = nc.dram_tensor("wk", [128, 8, 256], bf16, kind="ExternalInput").ap()
    wv = nc.dram_tensor("wv", [128, 8, 256], bf16, kind="ExternalInput").ap()
    wo = nc.dram_tensor("wo", [128, 8, 1024], bf16,
                        kind="ExternalInput").ap()
    out = nc.dram_tensor("out", [512, D], fp32, kind="ExternalOutput").ap()

    Exp = mybir.ActivationFunctionType.Exp

    with tile.TileContext(nc) as tc, ExitStack() as ctx:
        const = ctx.enter_context(tc.tile_pool(name="const", bufs=1))
        persist = ctx.enter_context(tc.tile_pool(name="persist", bufs=1))
        # PSUM: "ps" [128,1024] tiles (2 banks) serve projections AND
        # attention scores; "xps" [128,512] tiles (1 bank) serve the AV
        # accumulator and the output projection.  2*2 + 4*1 = 8 banks.
        spsum = ctx.enter_context(tc.tile_pool(name="spsum", bufs=2,
                                               space="PSUM"))
        xps = ctx.enter_context(tc.tile_pool(name="xps", bufs=4,
                                             space="PSUM"))
        ptile = ctx.enter_context(tc.tile_pool(name="ptile", bufs=5))
        xtp = ctx.enter_context(tc.tile_pool(name="xtp", bufs=3))
        xtnp = ctx.enter_context(tc.tile_pool(name="xtnp", bufs=5))
        misc = ctx.enter_context(tc.tile_pool(name="misc", bufs=2))
        outp = ctx.enter_context(tc.tile_pool(name="outp", bufs=2))

        # --- constants -----------------------------------------------------
        mask4 = const.tile([128, 4, 512], bf16, name="mask4")
        nc.gpsimd.memset(mask4[:], 1.0)
        for o in range(4):
            nc.gpsimd.affine_select(
                out=mask4[:, o, :], in_=mask4[:, o, :],
                compare_op=mybir.AluOpType.is_ge, fill=0.0, base=-128 * o,
                pattern=[[1, 512]], channel_multiplier=-1)

        wq_sb = const.tile([128, 8, 256], bf16, name="wq_sb")
        wk_sb = const.tile([128, 8, 256], bf16, name="wk_sb")
        wv_sb = const.tile([128, 8, 256], bf16, name="wv_sb")
        wo_sb = const.tile([128, 8, 1024], bf16, name="wo_sb")

        xqT_sb = persist.tile([128, 4, 8, 512], bf16, name="xqT_sb")
        xkT_sb = persist.tile([128, 4, 8, 512], bf16, name="xkT_sb")
        xvT_sb = persist.tile([128, 4, 8, 512], bf16, name="xvT_sb")

        qT = persist.tile([128, 2, S], bf16, name="qT")
        kT = persist.tile([128, 2, S], bf16, name="kT")
        v65 = persist.tile([128, 16, 4 * 65], bf16, name="v65")
        nc.gpsimd.memset(
            v65.rearrange("p t (h c) -> p t h c", c=65)[:, :, :, 64], 1.0)
        xall = persist.tile([128, H_PER_CORE, 8, 128], bf16, name="xall")
        gate = const.tile([1, 16], bf16, name="gate")

        # --- DMA loads ----------------------------------------------------
        def load_block(eng, dram_ap, dst, i):
            eng.dma_start(dst[:, i], dram_ap[i])

        nc.sync.dma_start(wq_sb[:], wq[:])
        nc.sync.dma_start(wk_sb[:], wk[:])
        load_block(nc.sync, xqT, xqT_sb, 0)
        load_block(nc.sync, xkT, xkT_sb, 0)
        nc.sync.dma_start(wv_sb[:], wv[:])
        load_block(nc.sync, xvT, xvT_sb, 0)
        # Gate: the scheduler may hoist dependency-free DMAs, so each bulk
        # load is held back by a REAL hazard: a 1-element gpsimd op reads
        # (a) the chain value, whose root RAW-depends on the last
        # first-block load, and (b) one element of the bulk load's own
        # destination region — the DMA write then has a WAR dependency and
        # cannot start before the first block has the HBM to itself.
        nc.gpsimd.tensor_copy(gate[:1, 0:1], xvT_sb[:1, 0, 0, 511:512])
        gi = [0]

        def gated(load_fn, dst_probe):
            gi[0] += 1
            nc.gpsimd.tensor_tensor(gate[:1, gi[0]:gi[0] + 1],
                                    gate[:1, 0:1], dst_probe,
                                    op=mybir.AluOpType.add)
            load_fn()

        for i in range(1, 4):
            for dram_ap, dst in ((xqT, xqT_sb), (xkT, xkT_sb),
                                 (xvT, xvT_sb)):
                gated(lambda d=dram_ap, t=dst, ii=i:
                      load_block(nc.gpsimd, d, t, ii),
                      dst[:1, i, 0, 0:1])
        gated(lambda: nc.gpsimd.dma_start(wo_sb[:], wo[:]),
              wo_sb[:1, 0, 0:1])

        # --- building blocks ----------------------------------------------
        def qkproj(w_sb, x_sb, dst, a, ic, tag):
            """dst[:, a, 512*ic:...] = (W[:, 128a:128(a+1)])^T @ x^T block."""
            ps = spsum.tile([128, 1024], fp32, tag="ps",
                            name=f"pp_{tag}_{a}_{ic}")
            for dc in range(8):
                nc.tensor.matmul(
                    ps[:, :512],
                    lhsT=w_sb[:, dc, 128 * a:128 * (a + 1)],
                    rhs=x_sb[:, ic, dc, :],
                    start=(dc == 0), stop=(dc == 7))
            nc.vector.tensor_copy(dst[:, a, 512 * ic:512 * (ic + 1)],
                                  ps[:, :512])

        def vproj(t):
            """v65[:, t, h*65:(h*65+64)] = x_v s-tile t @ Wv (natural)."""
            ps = spsum.tile([128, 1024], fp32, tag="ps", name=f"psv_{t}")
            for dc in range(8):
                nc.tensor.matmul(
                    ps[:, :256],
                    lhsT=xvT_sb[:, t // 4, dc,
                                128 * (t % 4):128 * (t % 4 + 1)],
                    rhs=wv_sb[:, dc, :],
                    start=(dc == 0), stop=(dc == 7))
            nc.vector.tensor_copy(
                v65.rearrange("p t (h c) -> p t h c", c=65)[:, t, :, :64],
                ps[:, :256].rearrange("p (h c) -> p h c", c=64))

        def attn_pair(a, ic, fillers):
            """Scores+exp+mask+AV for heads (2a, 2a+1), queries block ic.

            Scores are quadrant-packed: head sg in PE rows 64sg..64sg+63,
            column group c in PE cols 64c..64c+63 -> 4 concurrent MMs.
            One filler callback is drained after each b2 group so the PE
            has work while ACT churns through exp.
            Returns the two xtn tiles (transposed unnormalized x+denom).
            """
            nlive = 4 * (ic + 1)
            nbatch = nlive // 2
            px = [xps.tile([128, 512], fp32, tag="xps",
                           name=f"px_{a}_{ic}_{sg}") for sg in range(2)]
            pbs = [None] * nbatch
            xtns = []
            for b2 in range(nbatch + 1):
                if b2 < nbatch:
                    diag = 2 * b2 >= 4 * ic
                    pss = [spsum.tile([128, 1024], fp32, tag="ps",
                                      name=f"ps_{a}_{ic}_{b2}_{sg}")
                           for sg in range(2)]
                    qo0 = 0
                    for k2 in range(2):
                        jj = 2 * b2 + k2
                        o = jj - 4 * ic
                        qo = 128 * o if o > 0 else 0
                        if k2 == 0:
                            qo0 = qo
                        for sg in range(2):
                            po = 64 * sg
                            nc.tensor.matmul(
                                pss[sg][:, 512 * k2 + qo:512 * (k2 + 1)],
                                lhsT=kT[po:po + 64, a,
                                        128 * jj:128 * (jj + 1)],
                                rhs=qT[po:po + 64, a,
                                       512 * ic + qo:512 * (ic + 1)],
                                start=True, stop=True)
                    pbp = [ptile.tile([128, 2, 512], bf16, tag="pb",
                                      name=f"pb_{a}_{ic}_{b2}_{sg}")
                           for sg in range(2)]
                    for sg in range(2):
                        pb2d = pbp[sg].rearrange("p k f -> p (k f)")
                        # exp over [first live col of k2=0 .. end]; the
                        # dead middle columns hold stale-but-finite fp32,
                        # get multiplied to 0 by the mask, and the AV MMs
                        # never consume them.
                        nc.scalar.activation(pb2d[:, qo0:], pss[sg][:, qo0:],
                                             Exp, scale=SCALE)
                        if diag:
                            o0 = 2 * b2 - 4 * ic
                            nc.vector.tensor_mul(
                                pb2d[:, qo0:], pb2d[:, qo0:],
                                mask4[:, o0:o0 + 2, :].rearrange(
                                    "p k f -> p (k f)")[:, qo0:])
                    pbs[b2] = pbp
                if b2 >= 1:
                    for k2 in range(2):
                        jj = 2 * (b2 - 1) + k2
                        o = jj - 4 * ic
                        qo = 128 * o if o > 0 else 0
                        for sg in range(2):
                            h = 2 * a + sg
                            nc.tensor.matmul(
                                px[sg][:65, qo:],
                                lhsT=v65[:, jj, 65 * h:65 * (h + 1)],
                                rhs=pbs[b2 - 1][sg][:, k2, qo:],
                                start=(jj == 0), stop=(jj == nlive - 1))
                if fillers:
                    fillers.pop(0)()
            for sg in range(2):
                h = 2 * a + sg
                # x_unnorm^T (+denom row 64) -> natural via DMA xbar.
                # 80 = next multiple of XBAR_TILE_SRC_ROWS(16) above 65.
                xt_sb = xtp.tile([80, 512], bf16, tag="xt",
                                 name=f"xt_{h}_{ic}")
                nc.vector.tensor_copy(xt_sb[:65, :], px[sg][:65, :])
                xtn = xtnp.tile([128, 4, 80], bf16, tag="xtn",
                                name=f"xtn_{h}_{ic}")
                nc.sync.dma_start(xtn[:], xt_sb[:], transpose=True)
                xtns.append(xtn)
            return xtns

        def finalize(h, ic, xtn):
            """1/denom scatter into the buggy-merge layout of xall."""
            recip4 = misc.tile([128, 4], fp32, tag="recip4",
                               name=f"rc_{h}_{ic}")
            nc.vector.reciprocal(recip4[:], xtn[:, :, 64])
            for k4 in range(4):
                j = 4 * ic + k4
                nc.vector.tensor_scalar_mul(
                    xall[:, h, j % 8, (j // 8)::2],
                    xtn[:, k4, :64], recip4[:, k4:k4 + 1])

        def outproj(h):
            ot = outp.tile([128, 2, 512], fp32, tag="ot", name=f"ot_{h}")
            for nn in range(2):
                po_ = xps.tile([128, 512], fp32, tag="xps",
                               name=f"po_{h}_{nn}")
                for q8 in range(8):
                    nc.tensor.matmul(
                        po_[:],
                        lhsT=xall[:, h, q8, :],
                        rhs=wo_sb[:, q8, 512 * nn:512 * (nn + 1)],
                        start=(q8 == 0), stop=(q8 == 7))
                nc.vector.tensor_copy(ot[:, nn, :], po_[:])
            nc.sync.dma_start(out[128 * h:128 * (h + 1), :],
                              ot.rearrange("p k f -> p (k f)"))

        # === emission: PE queue order == program order.  Projections for
        # s-block ic+1, deferred finalizes and the output projection are
        # woven between the ACT-bound attention groups. ===================
        qkproj(wq_sb, xqT_sb, qT, 0, 0, "q")
        qkproj(wq_sb, xqT_sb, qT, 1, 0, "q")
        qkproj(wk_sb, xkT_sb, kT, 0, 0, "k")
        qkproj(wk_sb, xkT_sb, kT, 1, 0, "k")
        for t in range(4):
            vproj(t)

        # Deadline-aware filler quotas per (a, ic) slot.  qkproj(n) must be
        # fully emitted before pair(0, n) starts (its scores read qT/kT in
        # program order); vproj(t) before the AV matmuls of j-tile t
        # (earliest use: slot (0, t//4) at b2 >= 2*(t//4), so early slots
        # of that same pair are legal); outproj(h) after finalize(h, 3).
        def Q(w, x, dst, aa, n, tag):
            return lambda: qkproj(w, x, dst, aa, n, tag)

        def V(t):
            return lambda: vproj(t)

        slot_work = {
            (0, 0): [Q(wq_sb, xqT_sb, qT, 0, 1, "q"),
                     Q(wq_sb, xqT_sb, qT, 1, 1, "q"),
                     Q(wk_sb, xkT_sb, kT, 0, 1, "k")],
            (1, 0): [Q(wk_sb, xkT_sb, kT, 1, 1, "k"), V(4), V(5)],
            (0, 1): [V(6), V(7),
                     Q(wq_sb, xqT_sb, qT, 0, 2, "q"),
                     Q(wq_sb, xqT_sb, qT, 1, 2, "q"),
                     Q(wk_sb, xkT_sb, kT, 0, 2, "k")],
            (1, 1): [Q(wk_sb, xkT_sb, kT, 1, 2, "k"),
                     V(8), V(9), V(10), V(11)],
            (0, 2): [Q(wq_sb, xqT_sb, qT, 0, 3, "q"),
                     Q(wq_sb, xqT_sb, qT, 1, 3, "q"),
                     Q(wk_sb, xkT_sb, kT, 0, 3, "k"),
                     Q(wk_sb, xkT_sb, kT, 1, 3, "k")],
            (1, 2): [V(14), V(15)],
            (0, 3): [V(12), V(13)],
            (1, 3): [],
        }

        # Finalizes defer one pair-block normally; ic=0 blocks are so
        # short that their transpose+reciprocal would still sit on the
        # DVE critical path, so those defer two pair-blocks.
        pending = []  # [(h, ic, xtn, age), ...]
        for ic in range(4):
            for a in range(2):
                pending = [(h, i, x, age + 1) for (h, i, x, age) in pending]
                need = lambda p: p[3] >= (2 if p[1] == 0 else 1)
                drain = [p for p in pending if need(p)]
                pending = [p for p in pending if not need(p)]
                # Projection fillers go FIRST: their DVE casts are what the
                # next pair's scores wait on, and must precede the
                # latency-bound finalize chain (recip waits on a DMA
                # transpose) in the in-order DVE queue.
                fillers = list(slot_work[(a, ic)])
                for (hp, icp, xtnp_, _) in drain:
                    fillers.append(
                        lambda hp=hp, icp=icp, x=xtnp_: finalize(hp, icp, x))
                for (hp, icp, xtnp_, _) in drain:
                    if icp == 3:
                        fillers.append(lambda hp=hp: outproj(hp))
                xtns = attn_pair(a, ic, fillers)
                for f in fillers:  # leftovers (early ic: fewer b2 slots)
                    f()
                fillers.clear()
                pending.append((2 * a, ic, xtns[0], 0))
                pending.append((2 * a + 1, ic, xtns[1], 0))
        for (hp, icp, xtnp_, _) in pending:
            finalize(hp, icp, xtnp_)
            outproj(hp)

    nc.compile()
    return nc


def _get_nc():
    if "nc" not in _CACHE:
        _CACHE["nc"] = _build_kernel()
    return _CACHE["nc"]


def kernel(query, key, value, Wq, bq, Wk, bk, Wv, bv, Wo, bo):
    """Full inputs in, full output out. Shards batch x head-group over 8
    cores; host pre-transposes/casts so device DMA is pure bf16 HWDGE."""
    nc = _get_nc()
    from concourse.bass_utils import run_bass_kernel_spmd
    import ml_dtypes

    bf16 = ml_dtypes.bfloat16
    query = np.asarray(query, dtype=np.float32)
    key = np.asarray(key, dtype=np.float32)
    value = np.asarray(value, dtype=np.float32)
    Wq = np.asarray(Wq, dtype=np.float32)
    Wk = np.asarray(Wk, dtype=np.float32)
    Wv = np.asarray(Wv, dtype=np.float32)
    Wo = np.asarray(Wo, dtype=np.float32)

    # Pre-tile on host into the exact SBUF layouts so device DMAs are
    # fully linear: x^T -> [i, p, dc, s_local]; W -> [p, o, m].
    def tile_xT(x):  # x: [S, D] fp32 -> [4, 128, 8, 512] bf16
        xT = x.T.astype(bf16)  # [D, S]
        return np.ascontiguousarray(
            xT.reshape(8, 128, 4, 512).transpose(2, 1, 0, 3))

    def tile_w(w):  # w: [D, M] fp32 -> [128, 8, M] bf16
        m = w.shape[1]
        return np.ascontiguousarray(
            w.astype(bf16).reshape(8, 128, m).transpose(1, 0, 2))

    B = query.shape[0]
    xqT = [tile_xT(query[b]) for b in range(B)]
    xkT = [tile_xT(key[b]) for b in range(B)]
    xvT = [tile_xT(value[b]) for b in range(B)]
    wo_b = tile_w(Wo)

    in_maps = []
    for c in range(8):
        b, hg = c // 4, c % 4
        cols = slice(256 * hg, 256 * (hg + 1))
        in_maps.append({
            "xqT": xqT[b],
            "xkT": xkT[b],
            "xvT": xvT[b],
            "wq": tile_w(np.ascontiguousarray(Wq[:, cols])),
            "wk": tile_w(np.ascontiguousarray(Wk[:, cols])),
            "wv": tile_w(np.ascontiguousarray(Wv[:, cols])),
            "wo": wo_b,
        })

    trace = bool(int(os.environ.get("KERNEL_TRACE", "0")))
    res = run_bass_kernel_spmd(nc, in_maps, core_ids=list(range(8)),
                               trace=trace)
    _CACHE["last_result"] = res

    full = np.zeros((B, S, D), dtype=np.float32)
    for c in range(8):
        b, hg = c // 4, c % 4
        full[b, 512 * hg:512 * (hg + 1), :] = res.results[c]["out"]
    return full
